# revision 1
# baseline (speedup 1.0000x reference)
"""Trainium2 Bass kernel for DCNv2 block (deformable conv + BN + exact GELU).

Problem: x[8,768,32,32] -> offset/mask 3x3 convs -> deformable 3x3 conv
(768->768, bilinear sampling, sigmoid-mask modulation) -> BatchNorm(batch
stats) -> exact GELU.

Strategy (8 NeuronCores, data-parallel over batch, BN stats all-reduced):
  per core (1 image):
    1. fused offset+mask conv (27 out-ch) as 9 shifted matmuls on PE
    2. transpose conv output to j-on-partition layout; compute bilinear
       corner indices + weights with small DVE/ACT ops (clamping, border
       validity and the 2*sigmoid mask all folded into 4 slot weights)
    3. v-GEMM: v_k^T[j,co] = sum_c x[c,j] W[co,c,k]  (the full projection
       GEMM, done BEFORE sampling; bf16, fp32 PSUM) -> DRAM tables
    4. dma_gather (SWDGE) x-pair windows from the tables -> per-partition
       scalar FMA chains (scalar_tensor_tensor) -> identity-matmul into
       PSUM accumulates the 9 taps in fp32
    5. BN stats via ones-matmul, 8-core AllReduce, BN apply + exact GELU,
       write out^T [1024,768]; host reassembles to [8,768,32,32].
"""

import sys
import types

import numpy as np
import ml_dtypes

# Older axon client builds lack antenv.axon_hooks, which
# run_bass_kernel_spmd imports when tracing is requested via env. Stub it
# so the no-trace path always works standalone.
try:
    import antenv.axon_hooks  # noqa: F401
except ImportError:
    try:
        import antenv
        _stub = types.ModuleType("antenv.axon_hooks")
        _stub.get_axon_ntff_profile_hook = lambda: None
        sys.modules["antenv.axon_hooks"] = _stub
        antenv.axon_hooks = _stub
    except ImportError:
        pass

import concourse.bass as bass
import concourse.mybir as mybir
import concourse.tile as tile
from concourse import bacc
from concourse.masks import make_identity
from concourse.bass_utils import run_bass_kernel_spmd

FP32 = mybir.dt.float32
BF16 = mybir.dt.bfloat16
I16 = mybir.dt.int16
AF = mybir.ActivationFunctionType
OP = mybir.AluOpType

B, C, H, W = 8, 768, 32, 32
CO, KS = 768, 3
K9 = KS * KS
HWN = H * W          # 1024
CT = C // 128        # 6 c-tiles
NG = HWN // 128      # 8 j-groups
MAX_OFF = float(min(H, W) // 4)  # 8.0
THIRDS = [(0, 3), (3, 3), (6, 2)]  # (g0, ng) PSUM-sized j-group chunks

# idx relayout staging geometry
NLIST = K9 * 2            # 18 gather index lists (tap x y-corner)
LIST_STRIDE = 8192        # int16 elems per list in DRAM (64 rows of 128)
RELAY_NIDX = 1152         # 18*64 rows covered by the relayout transpose-gather
IDXDRAM_LEN = 163840      # 1280 rows of 128, padded for relayout overreads


def build_nc(n_cores: int, phase: int = 9, no_coll: bool = False):
    nc = bacc.Bacc(None, target_bir_lowering=False, debug=False)

    x_in = nc.dram_tensor("x_bf", [CT, 128, HWN], BF16, kind="ExternalInput")
    wconv_in = nc.dram_tensor("wconv", [CT, 128, K9, 27], BF16, kind="ExternalInput")
    wproj_in = nc.dram_tensor("wproj", [CT, 128, K9, CO], BF16, kind="ExternalInput")
    bias27_in = nc.dram_tensor("bias27", [27, 1], FP32, kind="ExternalInput")
    pb_in = nc.dram_tensor("pb", [128, CT], FP32, kind="ExternalInput")
    gamma_in = nc.dram_tensor("gamma", [128, CT], FP32, kind="ExternalInput")
    beta_in = nc.dram_tensor("beta", [128, CT], FP32, kind="ExternalInput")
    out_t = nc.dram_tensor("out", [HWN, CO], FP32, kind="ExternalOutput")

    from contextlib import ExitStack
    with tile.TileContext(nc) as tc:
        with ExitStack() as ctx:
            _emit(ctx, tc, n_cores, x_in, wconv_in, wproj_in, bias27_in, pb_in,
                  gamma_in, beta_in, out_t, phase, no_coll)
    nc.compile()
    return nc


def _emit(ctx, tc, n_cores, x_in, wconv_in, wproj_in, bias27_in, pb_in,
          gamma_in, beta_in, out_t, phase=9, no_coll=False):
    nc = tc.nc

    cpool = ctx.enter_context(tc.tile_pool(name="consts", bufs=1))
    wpool = ctx.enter_context(tc.tile_pool(name="weights", bufs=1))
    kpool = ctx.enter_context(tc.tile_pool(name="wk", bufs=2))
    spool = ctx.enter_context(tc.tile_pool(name="scratch", bufs=1))
    gpool = ctx.enter_context(tc.tile_pool(name="gather", bufs=4))
    tpool = ctx.enter_context(tc.tile_pool(name="tcomb", bufs=3))
    ypool = ctx.enter_context(tc.tile_pool(name="ybuf", bufs=1))
    opool = ctx.enter_context(tc.tile_pool(name="outb", bufs=2))
    dpool = ctx.enter_context(tc.tile_pool(name="dram", bufs=1, space="DRAM"))
    from contextlib import ExitStack
    early = ExitStack()
    pp_conv = early.enter_context(tc.tile_pool(name="ps_conv", bufs=1, space="PSUM"))
    pp_tp = early.enter_context(tc.tile_pool(name="ps_tp", bufs=2, space="PSUM"))

    # ---------------- constants ----------------
    ident_f = cpool.tile([128, 128], FP32, tag="identf", name="identf")
    make_identity(nc, ident_f[:, :])
    ident_b = cpool.tile([128, 128], BF16, tag="identb", name="identb")
    nc.vector.tensor_copy(ident_b[:, :], ident_f[:, :])
    ones_b = cpool.tile([128, 1], BF16, tag="onesb", name="onesb")
    nc.vector.memset(ones_b[:, :], 1.0)

    pf_i = cpool.tile([128, 1], mybir.dt.int32, tag="pfi", name="pfi")
    nc.gpsimd.iota(pf_i[:, :], pattern=[[0, 1]], base=0, channel_multiplier=1)
    pf = cpool.tile([128, 1], FP32, tag="pf", name="pf")
    nc.vector.tensor_copy(pf[:, :], pf_i[:, :])
    # hdiv = p // 32 (exact, via 3 compares); pm32 = p % 32
    hdiv = cpool.tile([128, 1], FP32, tag="hdiv", name="hdiv")
    tmp1 = cpool.tile([128, 1], FP32, tag="tmp1", name="tmp1")
    nc.vector.tensor_scalar(hdiv[:, :], pf[:, :], 32.0, None, OP.is_ge)
    nc.vector.tensor_scalar(tmp1[:, :], pf[:, :], 64.0, None, OP.is_ge)
    nc.vector.tensor_tensor(hdiv[:, :], hdiv[:, :], tmp1[:, :], OP.add)
    nc.vector.tensor_scalar(tmp1[:, :], pf[:, :], 96.0, None, OP.is_ge)
    nc.vector.tensor_tensor(hdiv[:, :], hdiv[:, :], tmp1[:, :], OP.add)
    pm32 = cpool.tile([128, 1], FP32, tag="pm32", name="pm32")
    nc.vector.scalar_tensor_tensor(pm32[:, :], hdiv[:, :], -32.0, pf[:, :],
                                   OP.mult, OP.add)

    kyrow_i = cpool.tile([128, K9], mybir.dt.int32, tag="kyrowi", name="kyrowi")
    nc.gpsimd.iota(kyrow_i[:, :].rearrange("p (a b) -> p a b", a=3),
                   pattern=[[1, 3], [0, 3]], base=0, channel_multiplier=0)
    kyrow = cpool.tile([128, K9], FP32, tag="kyrow", name="kyrow")
    nc.vector.tensor_copy(kyrow[:, :], kyrow_i[:, :])
    kxrow_i = cpool.tile([128, K9], mybir.dt.int32, tag="kxrowi", name="kxrowi")
    nc.gpsimd.iota(kxrow_i[:, :].rearrange("p (a b) -> p a b", a=3),
                   pattern=[[0, 3], [1, 3]], base=0, channel_multiplier=0)
    kxrow = cpool.tile([128, K9], FP32, tag="kxrow", name="kxrow")
    nc.vector.tensor_copy(kxrow[:, :], kxrow_i[:, :])

    # basex[p,k] = (p%32) + kx - 1   (same for every j-group)
    basex = cpool.tile([128, K9], FP32, tag="basex", name="basex")
    nc.vector.tensor_scalar(basex[:, :], kxrow[:, :], pm32[:, :], -1.0,
                            OP.add, OP.add)
    # basey[p,g,k] = (p//32) + 4g + ky - 1
    basey = cpool.tile([128, NG, K9], FP32, tag="basey", name="basey")
    for g in range(NG):
        nc.vector.tensor_scalar(basey[:, g, :], kyrow[:, :], hdiv[:, :],
                                float(4 * g - 1), OP.add, OP.add)

    # relayout-gather identity index list: value 16*s + (p % 16)
    # (dma_gather index lists must be replicated across 16-partition groups:
    #  each GPSIMD core reads the indices from its own group)
    pm16 = cpool.tile([128, 1], FP32, tag="pm16", name="pm16")
    nc.vector.tensor_scalar(pm16[:, :], pm32[:, :], 16.0, None, OP.is_ge)
    nc.vector.scalar_tensor_tensor(pm16[:, :], pm16[:, :], -16.0, pm32[:, :],
                                   OP.mult, OP.add)
    relay_i32 = cpool.tile([128, RELAY_NIDX // 16], mybir.dt.int32,
                           tag="relayi32", name="relayi32")
    nc.gpsimd.iota(relay_i32[:, :], pattern=[[16, RELAY_NIDX // 16]], base=0,
                   channel_multiplier=0)
    relay_f = cpool.tile([128, RELAY_NIDX // 16], FP32, tag="relayf", name="relayf")
    nc.vector.tensor_copy(relay_f[:, :], relay_i32[:, :])
    nc.vector.tensor_scalar(relay_f[:, :], relay_f[:, :], pm16[:, :], None, OP.add)
    relay_idx = cpool.tile([128, RELAY_NIDX // 16], I16, tag="relayidx", name="relayidx")
    nc.vector.tensor_copy(relay_idx[:, :], relay_f[:, :])

    # ---------------- load inputs ----------------
    x_sb = wpool.tile([128, CT, HWN], BF16, tag="xsb", name="xsb")
    nc.sync.dma_start(
        x_sb[:, :, :],
        x_in[:, :, :].rearrange("c p n -> p c n"))
    wconv_sb = wpool.tile([128, CT, K9, 27], BF16, tag="wconvsb", name="wconvsb")
    nc.sync.dma_start(
        wconv_sb[:, :, :, :],
        wconv_in[:, :, :, :].rearrange("c p k m -> p c k m"))
    bias27_sb = wpool.tile([27, 1], FP32, tag="bias27", name="bias27")
    nc.sync.dma_start(bias27_sb[:, :], bias27_in[:, :])
    pb_sb = wpool.tile([128, CT], FP32, tag="pbsb", name="pbsb")
    nc.sync.dma_start(pb_sb[:, :], pb_in[:, :])
    gamma_sb = wpool.tile([128, CT], FP32, tag="gammasb", name="gammasb")
    nc.sync.dma_start(gamma_sb[:, :], gamma_in[:, :])
    beta_sb = wpool.tile([128, CT], FP32, tag="betasb", name="betasb")
    nc.sync.dma_start(beta_sb[:, :], beta_in[:, :])

    # ---------------- offset/mask conv (27 out-ch) ----------------
    # zero-padded x (34x34) per c-tile for the conv windows
    xpad = wpool.tile([128, CT, 34 * 34], BF16, tag="xpad", name="xpad")
    nc.vector.memset(xpad[:, :, :], 0.0)
    for ct in range(CT):
        nc.vector.tensor_copy(
            xpad[:, ct, :].rearrange("p (h w) -> p h w", h=34)[:, 1:33, 1:33],
            x_sb[:, ct, :].rearrange("p (h w) -> p h w", h=H))
    xp_im = [xpad[:, ct, :].rearrange("p (h w) -> p h w", h=34) for ct in range(CT)]
    shifts = [(dy, dx) for dy in (-1, 0, 1) for dx in (-1, 0, 1)]
    offs_nat = spool.tile([27, HWN], FP32, tag="offsnat", name="offsnat")
    for half in range(2):
        conv_ps = pp_conv.tile([27, 512], FP32, tag="convps", name="convps")
        first = True
        for dy, dx in shifts:
            s = (dy + 1) * 3 + (dx + 1)
            for ct in range(CT):
                nc.tensor.matmul(
                    conv_ps[:, :],
                    wconv_sb[:, ct, s, :],
                    xp_im[ct][:, 1 + dy + 16 * half:1 + dy + 16 * half + 16,
                              1 + dx:1 + dx + 32],
                    start=first, stop=(dy, dx) == shifts[-1] and ct == CT - 1)
                first = False
        # evac + bias -> f32 natural layout
        nc.scalar.activation(
            offs_nat[:, 512 * half:512 * (half + 1)],
            conv_ps[:, :],
            AF.Identity, bias=bias27_sb[:, :])

    # transpose to j-on-partition: offT [128, g, 27]
    offT = spool.tile([128, NG, 27], FP32, tag="offT", name="offT")
    for g in range(NG):
        tp = pp_tp.tile([128, 27], FP32, tag="tpps", name="tpps")
        nc.tensor.transpose(tp[:, :], offs_nat[:, g * 128:(g + 1) * 128],
                            ident_f[:27, :27])
        nc.scalar.activation(offT[:, g, :], tp[:, :], AF.Copy)

    # ---------------- bilinear indices + weights ----------------
    # all [128, NG, K9] f32 unless noted
    def f3(tag):
        return spool.tile([128, NG, K9], FP32, tag=tag, name=tag)

    offy = offT[:, :, 0:18:2]
    offx = offT[:, :, 1:18:2]
    py = f3("py")
    px = f3("px")
    # clip offsets to +-MAX_OFF, add base position
    nc.vector.tensor_scalar(py[:, :, :], offy, -MAX_OFF, MAX_OFF, OP.max, OP.min)
    nc.vector.tensor_tensor(py[:, :, :], py[:, :, :], basey[:, :, :], OP.add)
    nc.vector.tensor_scalar(px[:, :, :], offx, -MAX_OFF, MAX_OFF, OP.max, OP.min)
    for g in range(NG):
        nc.vector.tensor_tensor(px[:, g, :], px[:, g, :], basex[:, :], OP.add)

    # robust floor via int cast (works for trunc or round-to-nearest)
    def fl(src, tag):
        t = f3(tag + "_t")
        nc.vector.tensor_scalar(t[:, :, :], src[:, :, :], 16.0, None, OP.add)
        ti = spool.tile([128, NG, K9], I16, tag=tag + "_i", name=tag + "_i")
        nc.vector.tensor_copy(ti[:, :, :], t[:, :, :])
        cf = f3(tag + "_cf")
        nc.vector.tensor_copy(cf[:, :, :], ti[:, :, :])
        over = f3(tag + "_ov")
        nc.vector.tensor_tensor(over[:, :, :], cf[:, :, :], t[:, :, :], OP.is_gt)
        nc.vector.tensor_tensor(cf[:, :, :], cf[:, :, :], over[:, :, :], OP.subtract)
        y0 = f3(tag + "_y0")
        nc.vector.tensor_scalar(y0[:, :, :], cf[:, :, :], 16.0, None, OP.subtract)
        fr = f3(tag + "_fr")
        nc.vector.tensor_tensor(fr[:, :, :], src[:, :, :], y0[:, :, :], OP.subtract)
        return y0, fr

    y0, fy = fl(py, "fy")
    x0, fx = fl(px, "fx")

    y0c = f3("y0c")
    nc.vector.tensor_scalar(y0c[:, :, :], y0[:, :, :], 0.0, 31.0, OP.max, OP.min)
    y1 = f3("y1")
    nc.vector.tensor_scalar(y1[:, :, :], y0[:, :, :], 1.0, None, OP.add)
    y1c = f3("y1c")
    nc.vector.tensor_scalar(y1c[:, :, :], y1[:, :, :], 0.0, 31.0, OP.max, OP.min)
    x0c = f3("x0c")
    nc.vector.tensor_scalar(x0c[:, :, :], x0[:, :, :], 0.0, 30.0, OP.max, OP.min)

    vy0 = f3("vy0")
    nc.vector.tensor_tensor(vy0[:, :, :], y0c[:, :, :], y0[:, :, :], OP.is_equal)
    vy1 = f3("vy1")
    nc.vector.tensor_tensor(vy1[:, :, :], y1c[:, :, :], y1[:, :, :], OP.is_equal)

    # x validity of corners A (x0) and B (x0+1)
    vxA = f3("vxA")
    t2 = f3("t2")
    nc.vector.tensor_scalar(vxA[:, :, :], x0[:, :, :], 0.0, None, OP.is_ge)
    nc.vector.tensor_scalar(t2[:, :, :], x0[:, :, :], 31.0, None, OP.is_le)
    nc.vector.tensor_tensor(vxA[:, :, :], vxA[:, :, :], t2[:, :, :], OP.mult)
    vxB = f3("vxB")
    nc.vector.tensor_scalar(vxB[:, :, :], x0[:, :, :], -1.0, None, OP.is_ge)
    nc.vector.tensor_scalar(t2[:, :, :], x0[:, :, :], 30.0, None, OP.is_le)
    nc.vector.tensor_tensor(vxB[:, :, :], vxB[:, :, :], t2[:, :, :], OP.mult)

    # slot coverage: slot0 = x0c, slot1 = x0c+1
    eqA = f3("eqA")
    nc.vector.tensor_tensor(eqA[:, :, :], x0c[:, :, :], x0[:, :, :], OP.is_equal)
    e0B = f3("e0B")
    nc.vector.tensor_scalar(t2[:, :, :], x0c[:, :, :], 1.0, None, OP.subtract)
    nc.vector.tensor_tensor(e0B[:, :, :], t2[:, :, :], x0[:, :, :], OP.is_equal)
    e1A = f3("e1A")
    nc.vector.tensor_scalar(t2[:, :, :], x0[:, :, :], 1.0, None, OP.subtract)
    nc.vector.tensor_tensor(e1A[:, :, :], x0c[:, :, :], t2[:, :, :], OP.is_equal)

    # a = (1-fx)*vxA ; b = fx*vxB
    wa = f3("wa")
    nc.vector.tensor_scalar(wa[:, :, :], fx[:, :, :], 1.0, -1.0, OP.subtract, OP.mult)
    nc.vector.tensor_tensor(wa[:, :, :], wa[:, :, :], vxA[:, :, :], OP.mult)
    wb = f3("wb")
    nc.vector.tensor_tensor(wb[:, :, :], fx[:, :, :], vxB[:, :, :], OP.mult)
    ws0 = f3("ws0")
    nc.vector.tensor_tensor(ws0[:, :, :], wa[:, :, :], eqA[:, :, :], OP.mult)
    nc.vector.tensor_tensor(t2[:, :, :], wb[:, :, :], e0B[:, :, :], OP.mult)
    nc.vector.tensor_tensor(ws0[:, :, :], ws0[:, :, :], t2[:, :, :], OP.add)
    ws1 = f3("ws1")
    nc.vector.tensor_tensor(ws1[:, :, :], wa[:, :, :], e1A[:, :, :], OP.mult)
    nc.vector.tensor_tensor(t2[:, :, :], wb[:, :, :], eqA[:, :, :], OP.mult)
    nc.vector.tensor_tensor(ws1[:, :, :], ws1[:, :, :], t2[:, :, :], OP.add)

    # y weights with 2*sigmoid(mask) folded in
    sig = f3("sig")
    nc.scalar.activation(sig[:, :, :], offT[:, :, 18:27], AF.Sigmoid)
    wy0 = f3("wy0")
    nc.vector.tensor_scalar(wy0[:, :, :], fy[:, :, :], 1.0, -2.0, OP.subtract, OP.mult)
    nc.vector.tensor_tensor(wy0[:, :, :], wy0[:, :, :], sig[:, :, :], OP.mult)
    nc.vector.tensor_tensor(wy0[:, :, :], wy0[:, :, :], vy0[:, :, :], OP.mult)
    wy1 = f3("wy1")
    nc.vector.tensor_scalar(wy1[:, :, :], fy[:, :, :], 2.0, None, OP.mult)
    nc.vector.tensor_tensor(wy1[:, :, :], wy1[:, :, :], sig[:, :, :], OP.mult)
    nc.vector.tensor_tensor(wy1[:, :, :], wy1[:, :, :], vy1[:, :, :], OP.mult)

    w00 = f3("w00")
    w01 = f3("w01")
    w10 = f3("w10")
    w11 = f3("w11")
    nc.vector.tensor_tensor(w00[:, :, :], wy0[:, :, :], ws0[:, :, :], OP.mult)
    nc.vector.tensor_tensor(w01[:, :, :], wy0[:, :, :], ws1[:, :, :], OP.mult)
    nc.vector.tensor_tensor(w10[:, :, :], wy1[:, :, :], ws0[:, :, :], OP.mult)
    nc.vector.tensor_tensor(w11[:, :, :], wy1[:, :, :], ws1[:, :, :], OP.mult)

    # flat indices (rows of the v tables), int16, staged [128, g, list]
    idlo = f3("idlo")
    nc.vector.scalar_tensor_tensor(idlo[:, :, :], y0c[:, :, :], 32.0,
                                   x0c[:, :, :], OP.mult, OP.add)
    idhi = f3("idhi")
    nc.vector.scalar_tensor_tensor(idhi[:, :, :], y1c[:, :, :], 32.0,
                                   x0c[:, :, :], OP.mult, OP.add)
    idfl = spool.tile([128, NG, NLIST], FP32, tag="idfl", name="idfl")
    nc.vector.tensor_copy(idfl[:, :, 0:NLIST:2], idlo[:, :, :])
    nc.vector.tensor_copy(idfl[:, :, 1:NLIST:2], idhi[:, :, :])
    # transpose to list-on-partition: S[l, 128g + p] = idx(p, g, l)
    s_f = spool.tile([NLIST, HWN], FP32, tag="sfidx", name="sfidx")
    for g in range(NG):
        tpx = pp_tp.tile([NLIST, 128], FP32, tag="tpx", name="tpx")
        nc.tensor.transpose(tpx[:, :], idfl[:, g, :], ident_f[:, :])
        nc.scalar.activation(s_f[:, g * 128:(g + 1) * 128], tpx[:, :], AF.Copy)
    s_i = spool.tile([NLIST, HWN], I16, tag="siidx", name="siidx")
    nc.vector.tensor_copy(s_i[:, :], s_f[:, :])

    if phase <= 1:
        early.close()
        return
    # ---------------- idx relayout through DRAM ----------------
    idxdram = dpool.tile([IDXDRAM_LEN], I16, tag="idxdram", name="idxdram")
    zt = spool.tile([128, IDXDRAM_LEN // 128], I16, tag="zeros", name="zeros")
    nc.vector.memset(zt[:, :], 0)
    nc.gpsimd.dma_start(idxdram[:IDXDRAM_LEN].rearrange("(a b) -> a b", a=128),
                        zt[:, :])
    # scatter stage -> flat j-order per list: dram[l*2304 + g*128 + p]
    # dram[(l*64 + s)*128 + 16*rep + r] = idx_l(16s + r) for rep 0..7:
    # each 256B row holds the 16 slot values replicated 8x so the relayout
    # gather lands replicated index rows on all 16-partition groups.
    _idxd = idxdram[:]
    for rep in range(8):
        dst = bass.AP(_idxd.tensor, 16 * rep,
                      [[LIST_STRIDE, NLIST], [128, 64], [1, 16]])
        nc.gpsimd.dma_start(dst, s_i[:, :])
    idxall = spool.tile([128, RELAY_NIDX], I16, tag="idxall", name="idxall")
    relay_src = idxdram[:].bitcast(BF16)
    relay_src = bass.AP(relay_src.tensor, relay_src.offset,
                        [[128, IDXDRAM_LEN // 128], [1, 128]])
    nc.gpsimd.dma_gather(
        out_ap=idxall[:, :].bitcast(BF16).rearrange("p (a b) -> p a b", a=1),
        in_ap=relay_src,
        idxs_ap=relay_idx[:, :],
        num_idxs=RELAY_NIDX,
        num_idxs_reg=RELAY_NIDX,
        elem_size=128,
        transpose=True,
        single_packet=False,
    )

    early.close()
    pp_v = ctx.enter_context(tc.tile_pool(name="ps_v", bufs=2, space="PSUM"))
    pp_o = ctx.enter_context(tc.tile_pool(name="ps_o", bufs=1, space="PSUM"))

    if phase <= 2:
        early.close()
        return
    # ---------------- v-GEMM -> DRAM tables + fused third-0 sampling -----
    vtabs = [dpool.tile([HWN, CO], BF16, tag=f"vtab{k}", name=f"vtab{k}") for k in range(K9)]
    y_bf = ypool.tile([128, NG, CO], BF16, tag="ybf", name="ybf")
    stats_sb = spool.tile([128, 2 * CT], FP32, tag="statssb", name="statssb")
    ops_tiles = {}

    def sample_third(it, k):
        """gathers + combine chains + identity-matmul accumulate for (third, tap)."""
        g0, ng = THIRDS[it]
        if k == 0:
            ops_tiles[it] = pp_o.tile([128, ng, 1024], FP32, tag="ops", name="ops")
        ops = ops_tiles[it]
        gb = []
        for yc in range(2):
            gt = gpool.tile([128, 3, 2 * CO], BF16, tag=f"gt{yc}", name=f"gt{yc}", bufs=3)
            lcol = (2 * k + yc) * 64 + 8 * g0
            vsrc = vtabs[k][:, :]
            vsrc = bass.AP(vsrc.tensor, vsrc.offset,
                           [[CO, HWN - 1], [1, 2 * CO]])
            nc.gpsimd.dma_gather(
                out_ap=gt[:, :ng, :],
                in_ap=vsrc,
                idxs_ap=idxall[:, lcol:lcol + 8 * ng],
                num_idxs=128 * ng,
                num_idxs_reg=128 * ng,
                elem_size=2 * CO,
                elem_step=CO,
                single_packet=False,
            )
            gb.append(gt)
        for gi in range(ng):
            g = g0 + gi
            t = tpool.tile([128, CO], BF16, tag="tcomb", name="tcomb", bufs=6)
            if it == 0:
                # v-GEMM span: PE-bound; keep chains compact on DVE (stt)
                nc.vector.tensor_scalar(t[:, :], gb[0][:, gi, 0:CO],
                                        w00[:, g, k:k + 1], None, OP.mult)
                nc.vector.scalar_tensor_tensor(
                    t[:, :], gb[0][:, gi, CO:2 * CO], w01[:, g, k:k + 1],
                    t[:, :], OP.mult, OP.add)
                nc.vector.scalar_tensor_tensor(
                    t[:, :], gb[1][:, gi, 0:CO], w10[:, g, k:k + 1],
                    t[:, :], OP.mult, OP.add)
                nc.vector.scalar_tensor_tensor(
                    t[:, :], gb[1][:, gi, CO:2 * CO], w11[:, g, k:k + 1],
                    t[:, :], OP.mult, OP.add)
            else:
                # tail: DVE-bound; ACT scales two corners, DVE adds (TT is 2x,
                # stt is 1x). Pool must stay free for gather DGE.
                t1 = tpool.tile([128, CO], BF16, tag="tc1", name="tc1", bufs=2)
                t2 = tpool.tile([128, CO], BF16, tag="tc2", name="tc2", bufs=2)
                nc.vector.tensor_scalar(t[:, :], gb[0][:, gi, 0:CO],
                                        w00[:, g, k:k + 1], None, OP.mult)
                nc.scalar.activation(t1[:, :], gb[0][:, gi, CO:2 * CO], AF.Copy,
                                     scale=w01[:, g, k:k + 1])
                nc.vector.tensor_tensor(t[:, :], t[:, :], t1[:, :], OP.add)
                nc.scalar.activation(t2[:, :], gb[1][:, gi, 0:CO], AF.Copy,
                                     scale=w10[:, g, k:k + 1])
                nc.vector.scalar_tensor_tensor(
                    t[:, :], gb[1][:, gi, CO:2 * CO], w11[:, g, k:k + 1],
                    t[:, :], OP.mult, OP.add)
                nc.vector.tensor_tensor(t[:, :], t[:, :], t2[:, :], OP.add)
            nc.tensor.matmul(ops[:, gi, 0:512], ident_b[:, :], t[:, 0:512],
                             start=k == 0, stop=k == K9 - 1)
            nc.tensor.matmul(ops[:, gi, 512:768], ident_b[:, :], t[:, 512:768],
                             start=k == 0, stop=k == K9 - 1)

    def finish_third(it, first):
        g0, ng = THIRDS[it]
        ops = ops_tiles[it]
        for gi in range(ng):
            g = g0 + gi
            nc.scalar.activation(y_bf[:, g, :], ops[:, gi, 0:768], AF.Copy)
            ysq = tpool.tile([128, CO], BF16, tag="ysq", name="ysq")
            nc.scalar.activation(ysq[:, :], y_bf[:, g, :], AF.Square)
            stats_g = pp_v.tile([128, 384], FP32, tag="vps", name="statsg")
            for cc in range(CT):
                nc.tensor.matmul(stats_g[:, cc:cc + 1],
                                 y_bf[:, g, cc * 128:(cc + 1) * 128],
                                 ones_b[:, :], start=True, stop=True)
                nc.tensor.matmul(stats_g[:, CT + cc:CT + cc + 1],
                                 ysq[:, cc * 128:(cc + 1) * 128],
                                 ones_b[:, :], start=True, stop=True)
            if first and gi == 0:
                nc.vector.tensor_copy(stats_sb[:, :], stats_g[:, 0:2 * CT])
            else:
                nc.vector.tensor_tensor(stats_sb[:, :], stats_sb[:, :],
                                        stats_g[:, 0:2 * CT], OP.add)

    for k in range(K9):
        wk = kpool.tile([128, CT, CO], BF16, tag="wk", name="wk")
        nc.sync.dma_start(wk[:, :, :],
                          wproj_in[:, :, k, :].rearrange("c p n -> p c n"))
        vstage = gpool.tile([128, NG, CO], BF16, tag="vstage", name="vstage",
                            bufs=2)
        for jc in range(NG):
            for half in range(2):
                ps = pp_v.tile([128, 384], FP32, tag="vps", name="vps")
                for ct in range(CT):
                    nc.tensor.matmul(
                        ps[:, :],
                        x_sb[:, ct, jc * 128:(jc + 1) * 128],
                        wk[:, ct, half * 384:(half + 1) * 384],
                        start=ct == 0, stop=ct == CT - 1)
                nc.scalar.activation(
                    vstage[:, jc, half * 384:(half + 1) * 384],
                    ps[:, :], AF.Copy)
        vdst = vtabs[k][:, :]
        vdst = bass.AP(vdst.tensor, vdst.offset,
                       [[CO, 128], [128 * CO, NG], [1, CO]])
        nc.sync.dma_start(vdst, vstage[:, :, :])
        # fuse third 0 of the sampling into the v-GEMM tap loop
        sample_third(0, k)

    if phase <= 3:
        early.close()
        return
    finish_third(0, first=True)
    for it in range(1, len(THIRDS)):
        for k in range(K9):
            sample_third(it, k)
        finish_third(it, first=False)

    # ---------------- BN stats all-reduce + coefficients ----------------
    cc_in = dpool.tile([128, 2 * CT], FP32, tag="ccin", name="ccin")
    cc_out = dpool.tile([128, 2 * CT], FP32, tag="ccout", name="ccout")
    nc.gpsimd.dma_start(cc_in[:, :], stats_sb[:, :])
    if no_coll:
        nc.gpsimd.dma_start(cc_out[:, :], cc_in[:, :])
    else:
        nc.gpsimd.collective_compute(
            "AllReduce", OP.add,
            replica_groups=[list(range(n_cores))],
            ins=[cc_in[:, :].opt()],
            outs=[cc_out[:, :].opt()],
        )
    nc.gpsimd.dma_start(stats_sb[:, :], cc_out[:, :])

    n_inv = 1.0 / float(n_cores * HWN)
    bnc = spool.tile([128, 2 * CT], FP32, tag="bnc", name="bnc")
    mean = spool.tile([128, CT], FP32, tag="mean", name="mean")
    nc.vector.tensor_scalar(mean[:, :], stats_sb[:, 0:CT], n_inv, None, OP.mult)
    var = spool.tile([128, CT], FP32, tag="var", name="var")
    nc.vector.tensor_tensor(var[:, :], mean[:, :], mean[:, :], OP.mult)
    nc.vector.scalar_tensor_tensor(var[:, :], stats_sb[:, CT:2 * CT], n_inv,
                                   var[:, :], OP.mult, OP.subtract)
    scale = bnc[:, 0:CT]
    shift = bnc[:, CT:2 * CT]
    nc.vector.tensor_scalar(var[:, :], var[:, :], 1e-5, None, OP.add)
    nc.vector.reciprocal(scale, var[:, :])
    nc.scalar.sqrt(scale, scale)
    nc.vector.tensor_tensor(scale, scale, gamma_sb[:, :], OP.mult)
    # proj_b cancels exactly under batch-stats BN: shift = beta - mean*scale
    nc.vector.tensor_tensor(shift, mean[:, :], scale, OP.mult)
    nc.vector.tensor_tensor(shift, beta_sb[:, :], shift, OP.subtract)

    # transpose [128, 12] -> [12, 128], then broadcast rows to full tiles
    bnt_ps = pp_v.tile([128, 384], FP32, tag="vps", name="bntps")
    nc.tensor.transpose(bnt_ps[:2 * CT, 0:128], bnc[:, :], ident_f[:, :])
    bnt = spool.tile([2 * CT, 128], BF16, tag="bnt", name="bnt")
    nc.scalar.activation(bnt[:, :], bnt_ps[:2 * CT, 0:128], AF.Copy)
    bnt0 = spool.tile([1, 2 * CT * 128], BF16, tag="bnt0", name="bnt0")
    nc.gpsimd.dma_start(
        bnt0[:, :].rearrange("q (r c) -> q r c", r=2 * CT), bnt[:, :])
    scsh = spool.tile([128, 2 * CT * 128], BF16, tag="scsh", name="scsh")
    nc.gpsimd.partition_broadcast(scsh[:, :], bnt0[:1, :])
    sc_b = scsh[:, 0:CO]
    sh_b = scsh[:, CO:2 * CO]

    # ---------------- BN apply + GELU + out ----------------
    for g in range(NG):
        yb = tpool.tile([128, CO], BF16, tag="yapply", name="yapply")
        nc.vector.tensor_tensor(yb[:, :], y_bf[:, g, :], sc_b[:, :], OP.mult)
        nc.vector.tensor_tensor(yb[:, :], yb[:, :], sh_b[:, :], OP.add)
        og = opool.tile([128, CO], FP32, tag="og", name="og")
        nc.scalar.activation(og[:, :], yb[:, :], AF.Gelu)
        nc.sync.dma_start(out_t[g * 128:(g + 1) * 128, :], og[:, :])


_NC_CACHE = {}


def _get_nc(n_cores):
    if n_cores not in _NC_CACHE:
        _NC_CACHE[n_cores] = build_nc(n_cores)
    return _NC_CACHE[n_cores]


def prep_inputs(x, proj_w, proj_b, off_w, off_b, mask_w, mask_b, gamma, beta):
    """Build the per-core in_maps (host-side layout prep only)."""
    bf = ml_dtypes.bfloat16
    wconv = np.concatenate([np.asarray(off_w), np.asarray(mask_w)], axis=0)
    wconv = wconv.reshape(27, CT, 128, K9).transpose(1, 2, 3, 0).astype(bf)
    wproj = np.asarray(proj_w).reshape(CO, CT, 128, K9).transpose(1, 2, 3, 0).astype(bf)
    bias27 = np.concatenate([np.asarray(off_b), np.asarray(mask_b)]).reshape(27, 1)
    bias27 = np.ascontiguousarray(bias27, dtype=np.float32)
    pb = np.ascontiguousarray(np.asarray(proj_b).reshape(CT, 128).T, np.float32)
    ga = np.ascontiguousarray(np.asarray(gamma).reshape(CT, 128).T, np.float32)
    be = np.ascontiguousarray(np.asarray(beta).reshape(CT, 128).T, np.float32)
    xs = np.asarray(x).reshape(B, CT, 128, HWN).astype(bf)
    in_maps = []
    for b in range(B):
        in_maps.append({
            "x_bf": np.ascontiguousarray(xs[b]),
            "wconv": wconv, "wproj": wproj, "bias27": bias27,
            "pb": pb, "gamma": ga, "beta": be,
        })
    return in_maps


def kernel(x, proj_w, proj_b, off_w, off_b, mask_w, mask_b, gamma, beta,
           _trace=False):
    n_cores = B
    nc = _get_nc(n_cores)
    in_maps = prep_inputs(x, proj_w, proj_b, off_w, off_b, mask_w, mask_b,
                          gamma, beta)
    res = run_bass_kernel_spmd(nc, in_maps, core_ids=list(range(n_cores)),
                               trace=_trace)
    outs = res.results if hasattr(res, "results") else res
    out = np.stack([np.asarray(outs[b]["out"]) for b in range(B)], axis=0)
    # [B, HW, CO] -> [B, CO, H, W] (pure layout, part of unshard)
    full = out.transpose(0, 2, 1).reshape(B, CO, H, W).astype(np.float32)
    if _trace:
        return full, res
    return full

